# revision 5
# baseline (speedup 1.0000x reference)
"""Trainium2 Bass kernel for nn_MultiHeadMixer — optimized v2 (bf16).

Same sharding as v1: core c = (batch b=c//2, head-group hg=c%2 of 8 heads,
embed cols 512*hg..512*hg+512).  Host sums the two head-group partials.

Changes vs v1:
  - mixer: q-outer with interleaved par col-strip pairs (overlap on the PE
    col groups); one mixer PSUM bank live at a time
  - no PE bias preload: epilogue is DVE (y2 = ps*invX, then y2 += biasX
    in place) with host-built inv/bias broadcast panels
  - proj2 computed transposed: out2[t,e'] with lhsT = y2 (stationary reused
    across e'-halves), so the host sums partials with no transpose
  - output stores alternate between both DMA rings; drains alternate DVE/ACT
"""

import contextlib

import numpy as np
import ml_dtypes

import concourse.bass as bass
import concourse.bacc as bacc
import concourse.mybir as mybir
import concourse.tile as tile
from concourse.bass_types import AP
from concourse.bass_utils import run_bass_kernel_spmd

B, S, E, H = 4, 2048, 1024, 16
D = E // H
N_CORES = 8
HPC = 8          # heads per core
CPC = 512        # embed cols per core
SB = S // 128    # 16 s-blocks
EB = E // 128    # 8 e-blocks
TQ = S // 512    # 4 t-quads

BF16 = mybir.dt.bfloat16
F32 = mybir.dt.float32
NPBF16 = ml_dtypes.bfloat16

_CACHED = {}


def emit_body(nc, tc, aps, P):
    xT, w1T, b1x, w2T, Tps, invX, bX, out2 = aps
    (xt_pool, w1_pool, w2_pool, tp_pool, pan_pool, cmp_pool,
     xp_pool, y2_pool, ost_pool, ps_pool) = P

    # ---- loads ----
    w1_t = []
    for k in range(EB):
        w = w1_pool.tile([128, CPC], BF16, tag="w1", name=f"w1_{k}")
        nc.scalar.dma_start(w[:], w1T[128 * k:128 * (k + 1), :])
        w1_t.append(w)
    b1_t = cmp_pool.tile([128, CPC], F32, tag="b1", name="b1_t")
    nc.scalar.dma_start(b1_t[:], b1x[:])
    xt_t = []
    for k in range(EB):
        t = xt_pool.tile([128, S], BF16, tag="xt", name=f"xt_{k}")
        nc.sync.dma_start(t[:], xT[128 * k:128 * (k + 1), :])
        xt_t.append(t)
    tp_t = []
    for h in range(HPC):
        t = tp_pool.tile([128, 2048], BF16, tag="tp", name=f"tp_{h}")
        nc.scalar.dma_start(t[:], Tps[h])
        tp_t.append(t)
    w2_t = []
    for k in range(4):
        t = w2_pool.tile([128, E], BF16, tag="w2", name=f"w2_{k}")
        nc.scalar.dma_start(t[:], w2T[128 * k:128 * (k + 1), :])
        w2_t.append(t)

    # ---- inv/bias panels (host-built; partitions 0-63 = head 2hp,
    # 64-127 = head 2hp+1, matching the mixer PSUM halves) ----
    invX_t, bX_t = [], []
    for hp in range(4):
        iv = pan_pool.tile([128, S], BF16, tag="invX", name=f"invX_{hp}")
        nc.sync.dma_start(iv[:], invX[hp])
        bx = pan_pool.tile([128, S], BF16, tag="bX", name=f"bX_{hp}")
        nc.scalar.dma_start(bx[:], bX[hp])
        invX_t.append(iv)
        bX_t.append(bx)

    # ---- proj1: xp[s-blk][128, 512] ----
    xp_t = []
    for m in range(SB):
        ps = ps_pool.tile([128, CPC], F32, tag="ps", name=f"ps1_{m}")
        for k in range(EB):
            nc.tensor.matmul(
                ps[:],
                xt_t[k][:, 128 * m:128 * (m + 1)],
                w1_t[k][:],
                start=(k == 0),
                stop=(k == EB - 1),
            )
        xp = xp_pool.tile([128, CPC], BF16, tag="xp", name=f"xp_{m}")
        nc.vector.tensor_add(xp[:], ps[:], b1_t[:])
        xp_t.append(xp)

    # ---- mixer: q-outer, per-(hp,q) chain of interleaved par pairs ----
    y2_t = {}
    for q in range(TQ):
        n_i = 4 * q + 4
        tcol = slice(512 * q, 512 * (q + 1))
        for hp in range(4):
            ps = ps_pool.tile([128, CPC], F32, tag="ps", name=f"psm_{hp}_{q}")
            for i in range(n_i):
                for par in range(2):
                    h = 2 * hp + par
                    prow = slice(64 * par, 64 * par + 64)
                    off = 128 * (4 * q - i)
                    ncol0 = 128 * (i - 4 * q) if i > 4 * q else 0
                    nc.tensor.matmul(
                        ps[prow, ncol0:CPC],
                        xp_t[i][:, 64 * h:64 * (h + 1)],
                        tp_t[h][:, off + ncol0:off + CPC],
                        start=(i == 0),
                        stop=(i == n_i - 1),
                        skip_group_check=True,
                    )
            y2 = y2_pool.tile([128, CPC], BF16, tag="y2", name=f"y2_{hp}_{q}")
            nc.vector.tensor_mul(y2[:], ps[:], invX_t[hp][:, tcol])
            nc.vector.tensor_add(y2[:], y2[:], bX_t[hp][:, tcol])
            y2_t[(hp, q)] = y2

    # ---- proj2 (transposed): out2[t, e'] = sum_c y2[c, t] * w2T[c, e'] ----
    for q in range(TQ):
        pss2 = {}
        for tsub in range(4):
            for eh in range(2):
                pss2[(tsub, eh)] = ps_pool.tile(
                    [128, CPC], F32, tag="ps", name=f"ps2_{q}_{tsub}_{eh}")
        for k in range(4):
            for tsub in range(4):
                for eh in range(2):
                    nc.tensor.matmul(
                        pss2[(tsub, eh)][:],
                        y2_t[(k, q)][:, 128 * tsub:128 * (tsub + 1)],
                        w2_t[k][:, 512 * eh:512 * (eh + 1)],
                        start=(k == 0),
                        stop=(k == 3),
                    )
        for tsub in range(4):
            for eh in range(2):
                idx = 2 * tsub + eh
                ost = ost_pool.tile([128, CPC], BF16, tag="ost",
                                    name=f"ost_{q}_{tsub}_{eh}")
                if idx % 2 == 0:
                    nc.scalar.copy(ost[:], pss2[(tsub, eh)][:])
                else:
                    nc.vector.tensor_copy(ost[:], pss2[(tsub, eh)][:])
                row0 = 512 * q + 128 * tsub
                dst = out2[row0:row0 + 128, 512 * eh:512 * (eh + 1)]
                if idx % 2 == 0:
                    nc.sync.dma_start(dst, ost[:])
                else:
                    nc.scalar.dma_start(dst, ost[:])


def build_program(loop_n=None):
    nc = bacc.Bacc("TRN2", target_bir_lowering=False, debug=False,
                   num_devices=N_CORES)

    aps = (
        nc.dram_tensor("xT", [E, S], BF16, kind="ExternalInput").ap(),
        nc.dram_tensor("w1T", [E, CPC], BF16, kind="ExternalInput").ap(),
        nc.dram_tensor("b1x", [128, CPC], F32, kind="ExternalInput").ap(),
        nc.dram_tensor("w2T", [CPC, E], BF16, kind="ExternalInput").ap(),
        nc.dram_tensor("Tps", [HPC, 128, 2048], BF16, kind="ExternalInput").ap(),
        nc.dram_tensor("invX", [4, 128, S], BF16, kind="ExternalInput").ap(),
        nc.dram_tensor("bX", [4, 128, S], BF16, kind="ExternalInput").ap(),
        nc.dram_tensor("out2", [S, E], BF16, kind="ExternalOutput").ap(),
    )

    with tile.TileContext(nc) as tc:
        with (
            tc.tile_pool(name="xt", bufs=EB) as xt_pool,
            tc.tile_pool(name="w1", bufs=EB) as w1_pool,
            tc.tile_pool(name="w2", bufs=4) as w2_pool,
            tc.tile_pool(name="tp", bufs=HPC) as tp_pool,
            tc.tile_pool(name="pan", bufs=4) as pan_pool,
            tc.tile_pool(name="cmp", bufs=1) as cmp_pool,
            tc.tile_pool(name="xp", bufs=SB) as xp_pool,
            tc.tile_pool(name="y2", bufs=16) as y2_pool,
            tc.tile_pool(name="ost", bufs=4) as ost_pool,
            tc.tile_pool(name="ps", bufs=8, space="PSUM") as ps_pool,
        ):
            P = (xt_pool, w1_pool, w2_pool, tp_pool, pan_pool, cmp_pool,
                 xp_pool, y2_pool, ost_pool, ps_pool)
            with (tc.For_i(0, loop_n, 1) if loop_n else contextlib.nullcontext()):
                emit_body(nc, tc, aps, P)

    nc.compile()
    return nc


def host_prep(x, weight, bias, inp_w, inp_b, out_w):
    x = np.asarray(x, np.float32)
    weight = np.asarray(weight, np.float32)
    bias = np.asarray(bias, np.float32)
    inp_w = np.asarray(inp_w, np.float32)
    inp_b = np.asarray(inp_b, np.float32)
    out_w = np.asarray(out_w, np.float32)

    norm = np.cumsum(weight, axis=1)            # (H, S)
    invn = (1.0 / norm).astype(np.float32)

    xT_b = [np.ascontiguousarray(x[b].T).astype(NPBF16) for b in range(B)]

    hg_pack = []
    for hg in range(2):
        heads = range(HPC * hg, HPC * hg + HPC)
        cols = slice(CPC * hg, CPC * hg + CPC)
        w1T = np.ascontiguousarray(inp_w[cols, :].T).astype(NPBF16)
        b1x = np.broadcast_to(inp_b[cols], (128, CPC)).astype(np.float32).copy()
        w2T = np.ascontiguousarray(out_w[:, cols].T).astype(NPBF16)
        Tps = np.zeros((HPC, 128, 2048), NPBF16)
        for hi, h in enumerate(heads):
            wrow = weight[h]
            for p in range(128):
                Tps[hi, p, p:2048] = wrow[:2048 - p]
        invX = np.zeros((4, 128, S), NPBF16)
        bX = np.zeros((4, 128, S), NPBF16)
        for hp in range(4):
            h0 = HPC * hg + 2 * hp
            invX[hp, :64] = invn[h0]
            invX[hp, 64:] = invn[h0 + 1]
            bX[hp, :64] = bias[h0].astype(NPBF16)
            bX[hp, 64:] = bias[h0 + 1].astype(NPBF16)
        hg_pack.append(dict(w1T=w1T, b1x=b1x, w2T=w2T, Tps=Tps,
                            invX=invX, bX=bX))

    in_maps = []
    for c in range(N_CORES):
        b, hg = c // 2, c % 2
        m = dict(hg_pack[hg])
        m["xT"] = xT_b[b]
        in_maps.append(m)
    return in_maps


def kernel(x, weight, bias, inp_w, inp_b, out_w):
    if "nc" not in _CACHED:
        _CACHED["nc"] = build_program()
    nc = _CACHED["nc"]

    in_maps = host_prep(x, weight, bias, inp_w, inp_b, out_w)
    res = run_bass_kernel_spmd(nc, in_maps, core_ids=list(range(N_CORES)))

    out = np.empty((B, S, E), np.float32)
    for b in range(B):
        p0 = np.asarray(res.results[2 * b]["out2"], dtype=np.float32)
        p1 = np.asarray(res.results[2 * b + 1]["out2"], dtype=np.float32)
        out[b] = p0 + p1
    return out
